# revision 7
# baseline (speedup 1.0000x reference)
"""Bass/Tile TRN2 kernel for per-token multi-head attention over heads.

Reference computation (per token t):
  qkv = x @ w_qkv + b_qkv                  # [t, 3072]
  q/k/v[h, d] = qkv[h*192 + {0,64,128} + d]
  scores[h, g] = q[h] . k[g] / 8
  attn = softmax(scores, axis=g)
  out[h, d] = sum_g attn[h, g] v[g, d]
  y = out.reshape(1024) @ w_out + b_out

Sharding: tokens (B*S = 32768) split evenly over 8 cores; weights replicated.
"""

import numpy as np
import ml_dtypes

H, DH = 16, 64
E = 1024
F3 = 3072
B, S = 4, 8192
N_CORES = 8
TOKS = (B * S) // N_CORES  # 4096 tokens per core
T = 256                    # tokens per unrolled iteration
NG = T // 8                # 8-token groups per iteration

NEG = -1.0e9


def _consts():
    """Host-built constant tensors embedded in the NEFF."""
    # scoresT layout: rows a=(t,g), cols b=(t',h); valid iff t==t'
    a = np.arange(128)
    mask = np.where((a[:, None] // 16) == (a[None, :] // 16), 0.0, NEG).astype(
        np.float32
    )
    ident = np.eye(128, dtype=np.float32)
    return mask, ident


def build(toks_per_core=TOKS):
    from concourse.bacc import Bacc
    import concourse.mybir as mybir
    from concourse.tile import TileContext
    from concourse.bass import ds

    f32 = mybir.dt.float32
    f32r = mybir.dt.float32r
    bf16 = mybir.dt.bfloat16
    niter = toks_per_core // T

    nc = Bacc("TRN2")
    x_d = nc.dram_tensor("x", [toks_per_core, E], f32r, kind="ExternalInput")
    wqkv_d = nc.dram_tensor("w_qkv", [E, F3], f32r, kind="ExternalInput")
    bqkv_d = nc.dram_tensor("b_qkv", [128, F3 // 128], f32, kind="ExternalInput")
    wout_d = nc.dram_tensor("w_out", [E, E], bf16, kind="ExternalInput")
    bout_d = nc.dram_tensor("b_out", [1, E], f32r, kind="ExternalInput")
    out_d = nc.dram_tensor("out", [toks_per_core, E], f32, kind="ExternalOutput")

    mask_np, ident_np = _consts()
    mask_c = nc.inline_tensor(mask_np, name="mask_c")
    identf_c = nc.inline_tensor(ident_np, name="identf_c")
    identb_c = nc.inline_tensor(ident_np.astype(ml_dtypes.bfloat16), name="identb_c")
    ones_c = nc.inline_tensor(np.ones((1, 128), np.float32), name="ones_c")

    with TileContext(nc) as tc:
        with (
            tc.tile_pool(name="persist", bufs=1) as pp,
            tc.tile_pool(name="xp", bufs=2) as xp,
            tc.tile_pool(name="xtp", bufs=1) as xtp,
            tc.tile_pool(name="qkvtp", bufs=4) as qkvtp,
            tc.tile_pool(name="packp", bufs=1) as packp,
            tc.tile_pool(name="attnsb", bufs=4) as attnsb,
            tc.tile_pool(name="stagep", bufs=1) as stagep,
            tc.tile_pool(name="outtokp", bufs=1) as outtokp,
            tc.tile_pool(name="outfp", bufs=2) as outfp,
            tc.tile_pool(name="psbig", bufs=4, space="PSUM") as psbig,
            tc.tile_pool(name="psattn", bufs=4, space="PSUM") as psattn,
        ):
            # ---- resident weights / constants ----
            w_sb = pp.tile([128, 8, F3], f32r)
            nc.sync.dma_start(w_sb, wqkv_d.rearrange("(ko kp) f -> kp ko f", kp=128))
            wout_sb = pp.tile([128, 8, E], bf16)
            nc.sync.dma_start(wout_sb, wout_d.rearrange("(ko kp) f -> kp ko f", kp=128))
            bqkv_sb = pp.tile([128, F3 // 128], f32)
            nc.sync.dma_start(bqkv_sb, bqkv_d[:])
            bout_sb = pp.tile([1, E], f32r)
            nc.sync.dma_start(bout_sb, bout_d[:])
            mask_sb = pp.tile([128, 128], f32)
            nc.sync.dma_start(mask_sb, mask_c[:])
            idf_sb = pp.tile([128, 128], f32r)
            nc.sync.dma_start(idf_sb, identf_c[:].bitcast(f32r))
            idb_sb = pp.tile([128, 128], bf16)
            nc.sync.dma_start(idb_sb, identb_c[:])
            ones_sb = pp.tile([1, 128], f32r)
            nc.sync.dma_start(ones_sb, ones_c[:].bitcast(f32r))

            for it in range(niter):
                t0 = it * T
                # ---- load x [T, E] as 2 sub-tiles of 128 tokens ----
                x_sb = xp.tile([128, T // 128, E], f32r)
                for jm in range(T // 128):
                    nc.sync.dma_start(
                        x_sb[:, jm, :], x_d[ds(t0 + jm * 128, 128), :]
                    )
                # ---- transpose x -> xT [e, t] ----
                xt_sb = xtp.tile([128, 8, T], f32r)
                for e in range(8):
                    for jm in range(T // 128):
                        pst = psattn.tile([128, 128], f32r, tag="ps_attn")
                        nc.tensor.transpose(
                            pst,
                            x_sb[:, jm, ds(e * 128, 128)],
                            idf_sb[:],
                        )
                        nc.any.tensor_copy(
                            out=xt_sb[:, e, ds(jm * 128, 128)],
                            in_=pst[:],
                        )

                # ---- QKV projection: qkvT tiles [128f x T] ----
                qkvt = []
                for j in range(F3 // 128):
                    psq_full = psbig.tile([128, 512], f32, tag="ps_big")
                    psq = psq_full[:, :T]
                    for e in range(8):
                        nc.tensor.matmul(
                            psq,
                            w_sb[:, e, ds(j * 128, 128)],
                            xt_sb[:, e, :],
                            start=(e == 0),
                            stop=(e == 7),
                        )
                    qt = qkvtp.tile([128, T], f32, tag="qkvt")
                    nc.scalar.activation(
                        qt[:],
                        psq,
                        mybir.ActivationFunctionType.Identity,
                        bias=bqkv_sb[:, j : j + 1],
                        scale=1.0,
                    )
                    qkvt.append(qt)

                # ---- repack q/k/v into [d, (t,h)] bf16 tiles ----
                qpack = packp.tile([64, T, 16], bf16, tag="qpack")
                kpack = packp.tile([64, T, 16], bf16, tag="kpack")
                vpack = packp.tile([65, T, 16], bf16, tag="vpack")
                nc.vector.memset(vpack[64:65, :, :], 1.0)

                def slab(frow):
                    jt, off = divmod(frow, 128)
                    return qkvt[jt][off : off + 64, :]

                for h in range(H):
                    nc.vector.tensor_copy(out=qpack[:, :, h], in_=slab(192 * h))
                    nc.vector.tensor_copy(
                        out=kpack[:, :, h], in_=slab(192 * h + 64)
                    )
                    nc.vector.tensor_copy(
                        out=vpack[:64, :, h], in_=slab(192 * h + 128)
                    )

                # ---- per 8-token group attention ----
                staging = stagep.tile([64, T, 16], bf16, tag="staging")
                for g in range(NG):
                    gs = ds(g * 8, 8)
                    # scoresT[(t,g), (t,h)] = k . q
                    psS = psattn.tile([128, 128], f32, tag="ps_attn")
                    nc.tensor.matmul(
                        psS,
                        kpack[:, gs, :].rearrange("p a b -> p (a b)"),
                        qpack[:, gs, :].rearrange("p a b -> p (a b)"),
                        start=True,
                        stop=True,
                    )
                    smask = attnsb.tile([128, 128], f32, tag="smask")
                    nc.vector.tensor_add(out=smask[:], in0=psS[:], in1=mask_sb[:])
                    expS = attnsb.tile([128, 128], bf16, tag="expS")
                    nc.scalar.activation(
                        expS[:],
                        smask[:],
                        mybir.ActivationFunctionType.Exp,
                        bias=0.0,
                        scale=0.125,
                    )
                    # vT_aug [(t,g), 65]: cols 0-63 = v, col 64 = ones
                    psV = psattn.tile([128, 65], bf16, tag="ps_attn")
                    nc.tensor.transpose(
                        psV,
                        vpack[:, gs, :].rearrange("p a b -> p (a b)"),
                        idb_sb[:65, :65],
                    )
                    vt_sb = attnsb.tile([128, 65], bf16, tag="vt")
                    nc.any.tensor_copy(out=vt_sb[:], in_=psV[:])
                    # AV: out[(t,h), 0:64] = sum_g expS * v ; col 64 = sum_g expS
                    psAV = psattn.tile([128, 65], f32, tag="ps_attn")
                    nc.tensor.matmul(
                        psAV, expS[:], vt_sb[:], start=True, stop=True
                    )
                    rec = attnsb.tile([128, 1], f32, tag="rec")
                    nc.vector.reciprocal(rec[:], psAV[:, 64:65])
                    onorm = attnsb.tile([128, 64], bf16, tag="onorm")
                    nc.vector.tensor_scalar(
                        out=onorm[:],
                        in0=psAV[:, 0:64],
                        scalar1=rec[:, 0:1],
                        scalar2=None,
                        op0=mybir.AluOpType.mult,
                    )
                    # transpose [(t,h), d] -> [d, (t,h)] and stash
                    psN = psattn.tile([64, 128], bf16, tag="ps_attn")
                    nc.tensor.transpose(psN, onorm[:], idb_sb[:])
                    nc.any.tensor_copy(
                        out=staging[:, gs, :].rearrange("p a b -> p (a b)"),
                        in_=psN[:],
                    )

                # ---- regroup to outtok [(h*64+d) chunks, t] ----
                outtok = outtokp.tile([128, 8, T], bf16, tag="outtok")
                for h in range(H):
                    nc.vector.tensor_copy(
                        out=outtok[(h % 2) * 64 : (h % 2) * 64 + 64, h // 2, :],
                        in_=staging[:, :, h],
                    )

                # ---- output projection + bias ----
                for jm in range(T // 128):
                    outf = outfp.tile([128, E], f32, tag="outf")
                    for nh in range(2):
                        psO = psbig.tile([128, 512], f32, tag="ps_big")
                        for k2 in range(8):
                            nc.tensor.matmul(
                                psO,
                                outtok[:, k2, ds(jm * 128, 128)],
                                wout_sb[:, k2, ds(nh * 512, 512)],
                                start=(k2 == 0),
                                stop=False,
                            )
                        nc.tensor.matmul(
                            psO,
                            ones_sb[:, :],
                            bout_sb[:, ds(nh * 512, 512)],
                            start=False,
                            stop=True,
                        )
                        nc.any.tensor_copy(
                            out=outf[:, ds(nh * 512, 512)], in_=psO[:]
                        )
                    nc.sync.dma_start(
                        out_d[ds(t0 + jm * 128, 128), :], outf[:]
                    )
    nc.finalize()
    return nc


_cache = {}


def _get_nc(toks_per_core=TOKS):
    if toks_per_core not in _cache:
        _cache[toks_per_core] = build(toks_per_core)
    return _cache[toks_per_core]


def prep_inputs(x, w_qkv, b_qkv, w_out, b_out, toks_per_core=TOKS, n_cores=N_CORES):
    """Shard tokens over cores; replicate (host-preprocessed) weights."""
    xf = np.ascontiguousarray(x, dtype=np.float32).reshape(-1, E)
    wq = np.ascontiguousarray(w_qkv, dtype=np.float32)
    bq = np.ascontiguousarray(
        np.asarray(b_qkv, dtype=np.float32).reshape(F3 // 128, 128).T
    )
    wo = np.ascontiguousarray(np.asarray(w_out).astype(ml_dtypes.bfloat16))
    bo = np.ascontiguousarray(np.asarray(b_out, dtype=np.float32).reshape(1, E))
    in_maps = []
    for c in range(n_cores):
        in_maps.append(
            {
                "x": np.ascontiguousarray(
                    xf[c * toks_per_core : (c + 1) * toks_per_core]
                ),
                "w_qkv": wq,
                "b_qkv": bq,
                "w_out": wo,
                "b_out": bo,
            }
        )
    return in_maps


def run(x, w_qkv, b_qkv, w_out, b_out, toks_per_core=TOKS, n_cores=N_CORES, **kw):
    from concourse import bass_utils

    nc = _get_nc(toks_per_core)
    in_maps = prep_inputs(
        x, w_qkv, b_qkv, w_out, b_out, toks_per_core, n_cores
    )
    res = bass_utils.run_bass_kernel_spmd(
        nc, in_maps, core_ids=list(range(n_cores)), **kw
    )
    out = np.concatenate([r["out"] for r in res.results], axis=0)
    return out, res


def kernel(x, w_qkv, b_qkv, w_out, b_out):
    out, _ = run(x, w_qkv, b_qkv, w_out, b_out)
    return out.reshape(x.shape[0], x.shape[1], E)
